# revision 4
# baseline (speedup 1.0000x reference)
"""Trainium2 Bass kernel for nn_CliquePotentialsCRF.

Math background
---------------
The reference runs MAX_ITER=100 Frank-Wolfe steps of
    g   = sigmoid(v + beta)
    s   = -alpha * energy_pool(g)
    gap = sum(g * (v - s));  done |= gap < TOL
    v   = v if done else v + 2/(t+2) * (s - v)
and returns -(beta + v).

With K=3, S=1 the energy pool is separable:
    energy_pool(X) = A @ X @ A - C ⊙ X        (per 128x128 image)
where A = W^T W, W the 126x128 sliding-window-sum operator, and
C = outer(diag A, diag A).

At t=0, gamma=1 so v1 = s0 = C⊙g0 - A g0 A.  At t=1 the "duality gap"
is large and NEGATIVE (~-54 for randn inputs), i.e. < TOL, so `done`
latches and v never changes again.  The output is therefore
    out = -(beta + v1) = A g0 A - C⊙g0 - beta,   g0 = sigmoid(beta).

The device computes exactly that (iteration 0).  The host then
verifies the freeze conditions numerically (gap0 >= TOL and
gap1 < TOL) in numpy; if they ever failed (never observed for this
input distribution), it falls back to an exact numpy continuation of
the full loop.

Sharding: pure data parallel.  B*C = 84 images -> padded to 88 -> 11
images per core on 8 cores, laid out [128 partitions, 11*128 cols].

Device pipeline (bf16 except PSUM accumulation):
  - ONE packed input dram tensor [A | -C | I | -beta], 2 input DMAs
    (fewer DMA instructions = much shorter end-of-NEFF DMA-drain tail)
  - g = sigmoid(-(-beta)) on scalar (activation scale=-1), 3 chunks
  - pass 1 (PE): pt_i = g_i^T A per image, PSUM
  - pt cast PSUM->SBUF bf16 per bank (vector)
  - nm = g * (-C) per bank (gpsimd), q = nm + (-beta) per bank (vector)
  - pass 2 (PE): z_i = pt_i^T A accumulating per bank; then one
    identity matmul per bank adds q: z_bank += I^T q_bank = q
    (so z ends up as A g A - C*g - beta with no extra DVE PSUM pass)
  - out cast PSUM->SBUF bf16 per bank on scalar (cheap PSUM reads,
    act-table for Copy warmed after the sigmoids)
  - ONE output DMA
"""

import os

import numpy as np
import ml_dtypes

N_CORES = 8
IMGS_PER_CORE = 11
H = 128
FD = IMGS_PER_CORE * H  # 1408
CONSTS = 3 * H  # A | -C | zeros
IN_FD = CONSTS + FD  # 1792
B, C_CH = 4, 21
N_IMGS = B * C_CH  # 84
TOL = 1e-3
ALPHA = 1.0
MAX_ITER = 100
PAD_BETA = -30000.0

# matmul / PSUM-bank groups: 4 + 4 + 3 images -> one 2KB bank each
GROUPS = [(0, 4), (4, 4), (8, 3)]

_bf16 = ml_dtypes.bfloat16


def _build_mats():
    """A = W^T W (symmetric banded), C = outer(diag A, diag A)."""
    W = np.zeros((H - 3 + 1, H), np.float32)
    for a in range(H - 2):
        W[a, a : a + 3] = 1.0
    A = (W.T @ W).astype(np.float32)
    cA = np.diag(A).copy()
    C = np.outer(cA, cA).astype(np.float32)
    return A, C


def _strip_spill_reload_dge(flags):
    """Drop the spill_reload DGE level: this kernel never spills, and each
    enabled DGE level adds per-engine semaphores that the NEFF epilogue
    resets one instruction at a time (~24 sems/engine for spill_reload)."""
    out = []
    i = 0
    while i < len(flags):
        f = flags[i]
        if f == "--internal-enable-dge-levels":
            out.append(f)
            i += 1
            while i < len(flags) and not flags[i].startswith("--"):
                if flags[i] != "spill_reload":
                    out.append(flags[i])
                i += 1
            continue
        out.append(f)
        i += 1
    return out


def _build_bass():
    from contextlib import ExitStack

    import concourse.mybir as mybir
    import concourse.tile as tile
    from concourse import bacc

    bf16 = mybir.dt.bfloat16
    f32 = mybir.dt.float32
    AF = mybir.ActivationFunctionType

    from concourse import compiler_utils

    flags = compiler_utils.get_compiler_flags()
    flags = _strip_spill_reload_dge(flags)
    compiler_utils.set_compiler_flags(flags)

    nc = bacc.Bacc("TRN2", target_bir_lowering=False, num_devices=N_CORES)
    # ONE packed input: [A | -C | I | -beta]
    inp_d = nc.dram_tensor("inp", [H, IN_FD], bf16, kind="ExternalInput")
    out_d = nc.dram_tensor("out", [H, FD], bf16, kind="ExternalOutput")

    with tile.TileContext(nc) as tc, ExitStack() as ctx:
        sb = ctx.enter_context(tc.tile_pool(name="sb", bufs=1))
        psum = ctx.enter_context(tc.tile_pool(name="psum", bufs=1, space="PSUM"))

        inp_sb = sb.tile([H, IN_FD], bf16, tag="inp")
        A_sb = inp_sb[:, 0:H]
        negC_sb = inp_sb[:, H : 2 * H]
        zero_col = inp_sb[:, 2 * H : 2 * H + 1]  # zeros block
        nbeta = inp_sb[:, CONSTS:]  # [-beta], [H, FD]

        # input DMAs: [consts | bank1] then [bank2 | bank3]
        split = CONSTS + 4 * H
        nc.sync.dma_start(inp_sb[:, 0:split], inp_d[:, 0:split])
        nc.sync.dma_start(inp_sb[:, split:], inp_d[:, split:])

        # act-table warmup: the table load has no data deps and is hoisted
        # to engine start; zero-column bias avoids const-AP memsets.
        warm = sb.tile([H, 1], bf16, tag="warm")
        nc.scalar.activation(warm[:], zero_col, AF.Sigmoid, bias=zero_col)

        g = sb.tile([H, FD], bf16, tag="g")
        pt_ps = psum.tile([H, 12 * H], f32, tag="pt")
        z_ps = psum.tile([H, 12 * H], f32, tag="z")
        pt_sb = sb.tile([H, FD], bf16, tag="pt_sb")
        nm = sb.tile([H, FD], bf16, tag="nm")
        q = sb.tile([H, FD], bf16, tag="q")
        out_sb = sb.tile([H, FD], bf16, tag="out")

        # sigmoids per bank: g = sigmoid(-( -beta ))
        for i0, ni in GROUPS:
            cols = slice(i0 * H, (i0 + ni) * H)
            nc.scalar.activation(
                g[:, cols], nbeta[:, cols], AF.Sigmoid, bias=zero_col, scale=-1.0
            )

        # pass 1 + per-bank epilogue prep
        for i0, ni in GROUPS:
            for s in range(ni):
                i = i0 + s
                nc.tensor.matmul(
                    pt_ps[:, i * H : (i + 1) * H],
                    g[:, i * H : (i + 1) * H],
                    A_sb,
                    start=True,
                    stop=True,
                )
            cols = slice(i0 * H, (i0 + ni) * H)
            # pt cast PSUM->SBUF bf16 (vector)
            nc.vector.tensor_scalar_add(pt_sb[:, cols], pt_ps[:, cols], 0.0)
            # nm = g * (-C)  (gpsimd, C broadcast across the bank's images)
            g_v = g[:, cols].rearrange("p (n m) -> p n m", n=ni)
            nm_v = nm[:, cols].rearrange("p (n m) -> p n m", n=ni)
            negC_bc = negC_sb[:, None, :].broadcast_to([H, ni, H])
            nc.gpsimd.tensor_mul(nm_v, g_v, negC_bc)
            # q = nm + (-beta)   (vector, bf16 SBUF = cheap)
            nc.vector.tensor_add(q[:, cols], nm[:, cols], nbeta[:, cols])

        # pass 2: z_i = pt_i^T A
        for i0, ni in GROUPS:
            for s in range(ni):
                i = i0 + s
                nc.tensor.matmul(
                    z_ps[:, i * H : (i + 1) * H],
                    pt_sb[:, i * H : (i + 1) * H],
                    A_sb,
                    start=True,
                    stop=True,
                )

        # out = z + q  (vector, reads PSUM)
        for i0, ni in GROUPS:
            cols = slice(i0 * H, (i0 + ni) * H)
            nc.vector.tensor_add(out_sb[:, cols], z_ps[:, cols], q[:, cols])

        nc.sync.dma_start(out_d[:], out_sb[:])

    nc.compile()
    return nc


def _energy_pool_np(x, A, C):
    # x: [n, H, H] float32
    return np.einsum("ki,nkl,lj->nij", A, x, A, optimize=True) - C[None] * x


def _fallback_loop(beta_imgs, v, A, C, t_start, done):
    """Exact numpy continuation of the reference loop from iteration t_start."""
    v = v.astype(np.float32).copy()
    for t in range(t_start, MAX_ITER):
        g = 1.0 / (1.0 + np.exp(-(v + beta_imgs)))
        s = -ALPHA * _energy_pool_np(g.astype(np.float32), A, C)
        gap = float(np.sum(g * (v - s), dtype=np.float64))
        done = done or (gap < TOL)
        gamma = np.float32(2.0 / (t + 2.0))
        if not done:
            v = v + gamma * (s - v)
    return v


def _run_device(beta):
    """Run the Bass SPMD kernel. Returns (out_imgs[84,H,H], results_obj)."""
    from concourse.bass_utils import run_bass_kernel_spmd

    A, C = _build_mats()
    imgs = beta.reshape(N_IMGS, H, H).astype(np.float32)
    n_pad = N_CORES * IMGS_PER_CORE - N_IMGS
    pad = np.full((n_pad, H, H), PAD_BETA, np.float32)
    imgs_p = np.concatenate([imgs, pad], axis=0)
    shards = imgs_p.reshape(N_CORES, IMGS_PER_CORE, H, H)

    consts = np.concatenate([A, -C, np.zeros((H, H), np.float32)], axis=1)
    in_maps = []
    for c in range(N_CORES):
        nb = -shards[c].transpose(1, 0, 2).reshape(H, FD)  # -beta, [128,1408]
        packed = np.ascontiguousarray(
            np.concatenate([consts, nb], axis=1).astype(_bf16)
        )  # [128, 1792] bf16
        in_maps.append({"inp": packed})

    nc = _build_bass()
    res = run_bass_kernel_spmd(
        nc,
        in_maps,
        core_ids=list(range(N_CORES)),
        trace_cores=list(range(N_CORES)) if os.environ.get("BASS_TRACE") else None,
    )

    outs = []
    for c in range(N_CORES):
        r = res.results[c]
        o = (
            r["out"]
            .astype(np.float32)
            .reshape(H, IMGS_PER_CORE, H)
            .transpose(1, 0, 2)
        )  # [11,H,H]
        outs.append(o)
    out_imgs = np.concatenate(outs, axis=0)[:N_IMGS]
    return out_imgs, res


def _host_gaps(beta_imgs, out_imgs, A, C):
    """gap0 and gap1 of the reference loop, from the device output.

    v1 = -out - beta;  gap0 = -sum(g0*v1);  gap1 = sum(g1*(v1 - s1)).
    """
    g0 = 1.0 / (1.0 + np.exp(-beta_imgs))
    v1 = -out_imgs - beta_imgs
    gap0 = -np.sum(g0 * v1, dtype=np.float64)
    g1 = (1.0 / (1.0 + np.exp(out_imgs))).astype(np.float32)  # sigmoid(v1+beta)
    s1 = -ALPHA * _energy_pool_np(g1, A, C)
    gap1 = float(np.sum(g1 * (v1 - s1), dtype=np.float64))
    return float(gap0), gap1, v1


def kernel(beta):
    beta = np.asarray(beta, dtype=np.float32)
    assert beta.shape == (B, C_CH, H, H), beta.shape

    out_imgs, _res = _run_device(beta)

    A, C = _build_mats()
    beta_i = beta.reshape(N_IMGS, H, H)
    gap0, gap1, v1 = _host_gaps(beta_i, out_imgs, A, C)

    if gap0 < TOL:
        # done latched before the first update: v stays 0
        return (-beta).astype(np.float32)

    if gap1 >= TOL:
        # loop did not freeze at t=1 -- exact numpy continuation from v1
        v = _fallback_loop(beta_i, v1, A, C, t_start=1, done=False)
        return (-(beta_i + v)).reshape(B, C_CH, H, H).astype(np.float32)

    return out_imgs.reshape(B, C_CH, H, H).astype(np.float32)
